# revision 19
# baseline (speedup 1.0000x reference)
"""Trainium2 Bass kernel for nn_BModel (BinaryLinear: out = x @ sign(W).T / sqrt(in_dim)).

Strategy (data-parallel over 8 NeuronCores):
  - x [4096, 32768] f32 is sharded along batch (512 rows/core) and
    host-marshalled (pure layout permutation, no arithmetic -- same category
    as the W.T transpose) into xh[bb, rh, p, j, b]: exactly the SBUF tile
    order the TensorEngine needs.  The device x-load is then FULLY
    contiguous (16-64 KB descriptor runs, 128 descriptors per tile instead
    of 16384), and the on-chip VectorE+ScalarE repack stage of the previous
    kernel disappears entirely -- matmuls read the DMA'd tile directly.
  - W [100, 32768] f32 is host-transposed to wt = W.T and replicated;
    sign() is computed on-device (ScalarE Sign from a bf16 cast, pre-scaled
    by 2^64; sign(0)=0 matches jnp.sign).
  - x tiles are loaded with a casting SWDGE DMA (f32 -> fp16); sign(W) is
    exact in fp16 and PSUM accumulates in f32, so the only error is fp16
    rounding of x (~2e-4 relative).
  - Matmuls: psum[c, b] += sum_p w_sT[p, c] * xr[p, j-chunk, b],
    accumulating over all 256 (rh, j) contraction chunks; evacuated with a
    fused 1/sqrt(K) scale on ScalarE; output is written transposed
    [100, B] and the host transposes it back.
"""

import math

import numpy as np

N_CORES = 8
BATCH = 4096
K = 32768
C = 100
P = 128  # SBUF partitions
J = 128  # k-chunks per rh half
RH = K // (P * J)  # 2
B_PER_CORE = BATCH // N_CORES  # 512

_NC_CACHE = {}


def _build_nc(b_per_core=B_PER_CORE, bn=128, xr_bufs=3):
    """Build + compile the per-core Bass program (identical on all cores)."""
    from contextlib import ExitStack

    import concourse.bass as bass
    import concourse.tile as tile
    from concourse import bacc, mybir

    f32 = mybir.dt.float32
    bf16 = mybir.dt.bfloat16
    f16 = mybir.dt.float16

    bb_count = b_per_core // bn

    nc = bacc.Bacc(
        "TRN2",
        target_bir_lowering=False,
        debug=False,
        num_devices=N_CORES,
    )

    xh = nc.dram_tensor(
        "xh", [bb_count, RH, P, J, bn], f32, kind="ExternalInput"
    ).ap()
    wh = nc.dram_tensor("wh", [16, P, 16, C], f32, kind="ExternalInput").ap()
    out_t = nc.dram_tensor("out_t", [C, b_per_core], f32, kind="ExternalOutput").ap()

    scale = 1.0 / math.sqrt(K)

    WJC = 16  # j-extent of one w chunk tile
    n_wchunks = (RH * J) // WJC

    with tile.TileContext(nc) as tc, ExitStack() as ctx:
        wpool = ctx.enter_context(tc.tile_pool(name="w", bufs=1))
        wtmp_pool = ctx.enter_context(tc.tile_pool(name="wtmp", bufs=2))
        xrpool = ctx.enter_context(tc.tile_pool(name="xr", bufs=xr_bufs))
        xqpool = ctx.enter_context(tc.tile_pool(name="xq", bufs=1))
        psum_pool = ctx.enter_context(tc.tile_pool(name="psum", bufs=2, space="PSUM"))
        opool = ctx.enter_context(tc.tile_pool(name="o", bufs=2))

        # --- W prep, emitted lazily so the first x tiles interleave with
        #     W-chunk loads.
        w_tiles = [None] * n_wchunks

        def emit_wchunk(t):
            wtmp = wtmp_pool.tile([P, WJC, C], bf16)
            # wh[t] is host-marshalled to the exact chunk-tile order, so this
            # casting DMA reads one fully contiguous 819 KB block.
            nc.gpsimd.dma_start(wtmp[:], wh[t])
            wtile = wpool.tile([P, WJC, C], f16, tag=f"w{t}")
            nc.scalar.activation(
                wtile[:],
                wtmp[:],
                mybir.ActivationFunctionType.Sign,
                scale=float(2.0**64),
            )
            w_tiles[t] = wtile

        pending_evac = []

        def emit_evac():
            psum_e, bb_e = pending_evac.pop(0)
            ot = opool.tile([C, bn], f32)
            nc.scalar.activation(
                ot[:], psum_e[:, :], mybir.ActivationFunctionType.Copy, scale=scale
            )
            nc.sync.dma_start(out_t[:, bb_e * bn : (bb_e + 1) * bn], ot[:])

        # --- main loop: per (bb, rh), one contiguous casting DMA (split into
        #     4 j-range sub-DMAs for pipelining) straight into the matmul
        #     layout; no repack stage at all.
        JSUB = 4
        JQ = J // JSUB
        for bb in range(bb_count):
            psum = psum_pool.tile([C, bn], f32)
            for rh in range(RH):
                last = bb == bb_count - 1 and rh == RH - 1
                if last:
                    # the final tile loads as 4 SEPARATE j-quarter tiles, so
                    # each quarter's matmuls start as soon as ITS quarter
                    # lands -- only the last 32 pairs remain after the
                    # stream ends.
                    xqs = [
                        xqpool.tile([P, JQ, bn], f16, name=f"xq{s}", tag=f"xq{s}")
                        for s in range(JSUB)
                    ]
                    for s in range(JSUB):
                        nc.gpsimd.dma_start(
                            xqs[s][:],
                            xh[bb, rh, :, s * JQ : (s + 1) * JQ, :],
                        )
                    for j in range(J):
                        t = (rh * J + j) // WJC
                        nc.tensor.matmul(
                            psum[:, :],
                            w_tiles[t][:, j % WJC, :],
                            xqs[j // JQ][:, j % JQ, :],
                            start=False,
                            stop=(j == J - 1),
                        )
                    continue
                xr = xrpool.tile([P, J, bn], f16, name="xr", tag="xr")
                for s in range(JSUB):
                    j0 = s * J // JSUB
                    j1 = (s + 1) * J // JSUB
                    nc.gpsimd.dma_start(
                        xr[:, j0:j1, :],
                        xh[bb, rh, :, j0:j1, :],
                    )
                    sub_idx = (bb * RH + rh) * JSUB + s
                    if sub_idx < 8:
                        for t2 in range(sub_idx * 2, sub_idx * 2 + 2):
                            emit_wchunk(t2)
                for j in range(J):
                    t = (rh * J + j) // WJC
                    nc.tensor.matmul(
                        psum[:, :],
                        w_tiles[t][:, j % WJC, :],
                        xr[:, j, :],
                        start=(rh == 0 and j == 0),
                        stop=(rh == RH - 1 and j == J - 1),
                    )
            # evacuate with one-bb lag so the (in-order) ScalarE queue never
            # head-of-line-blocks behind this bb's matmuls.
            pending_evac.append((psum, bb))
            if len(pending_evac) > 1:
                emit_evac()
        while pending_evac:
            emit_evac()

    nc.compile()
    return nc


def _get_nc(b_per_core=B_PER_CORE, bn=128, xr_bufs=3):
    key = (b_per_core, bn, xr_bufs)
    if key not in _NC_CACHE:
        _NC_CACHE[key] = _build_nc(*key)
    return _NC_CACHE[key]


def kernel(x, W, **run_kwargs):
    from concourse import bass_utils

    x = np.asarray(x, dtype=np.float32)
    W = np.asarray(W, dtype=np.float32)
    # pure layout permutation of W.T into chunk-tile order:
    # wh[t][p, jj, c] = W.T[rh_t*(P*J) + p*J + j0_t + jj, c]
    wq = np.ascontiguousarray(W.T).reshape(RH, P, 8, 16, C)
    wh = np.ascontiguousarray(wq.transpose(0, 2, 1, 3, 4)).reshape(16, P, 16, C)

    # pure layout permutation: xh[c][bb, rh, p, j, b] = x[c*512+bb*128+b,
    # rh*(P*J) + p*J + j] -- the exact SBUF tile order, so device loads are
    # fully contiguous.
    bb_count = B_PER_CORE // 128
    x6 = x.reshape(N_CORES, bb_count, 128, RH, P, J)
    xh = np.ascontiguousarray(x6.transpose(0, 1, 3, 4, 5, 2))

    nc = _get_nc()
    in_maps = [{"xh": xh[c], "wh": wh} for c in range(N_CORES)]
    res = bass_utils.run_bass_kernel_spmd(
        nc, in_maps, core_ids=list(range(N_CORES)), **run_kwargs
    )
    out = np.concatenate([r["out_t"].T for r in res.results], axis=0)
    if run_kwargs:
        return out, res
    return out


# revision 20
# speedup vs baseline: 1.0319x; 1.0319x over previous
"""Trainium2 Bass kernel for nn_BModel (BinaryLinear: out = x @ sign(W).T / sqrt(in_dim)).

Strategy (data-parallel over 8 NeuronCores):
  - x [4096, 32768] f32 is sharded along batch (512 rows/core) and
    host-marshalled (pure layout permutation, no arithmetic -- same category
    as the W.T transpose) into xh[bb, rh, p, j, b]: exactly the SBUF tile
    order the TensorEngine needs.  The device x-load is then FULLY
    contiguous (16-64 KB descriptor runs, 128 descriptors per tile instead
    of 16384), and the on-chip VectorE+ScalarE repack stage of the previous
    kernel disappears entirely -- matmuls read the DMA'd tile directly.
  - W [100, 32768] f32 is host-transposed to wt = W.T and replicated;
    sign() is computed on-device (ScalarE Sign from a bf16 cast, pre-scaled
    by 2^64; sign(0)=0 matches jnp.sign).
  - x tiles are loaded with a casting SWDGE DMA (f32 -> fp16); sign(W) is
    exact in fp16 and PSUM accumulates in f32, so the only error is fp16
    rounding of x (~2e-4 relative).
  - Matmuls: psum[c, b] += sum_p w_sT[p, c] * xr[p, j-chunk, b],
    accumulating over all 256 (rh, j) contraction chunks; evacuated with a
    fused 1/sqrt(K) scale on ScalarE; output is written transposed
    [100, B] and the host transposes it back.
"""

import math

import numpy as np

N_CORES = 8
BATCH = 4096
K = 32768
C = 100
P = 128  # SBUF partitions
J = 128  # k-chunks per rh half
RH = K // (P * J)  # 2
B_PER_CORE = BATCH // N_CORES  # 512

_NC_CACHE = {}


def _build_nc(b_per_core=B_PER_CORE, bn=128, xr_bufs=3):
    """Build + compile the per-core Bass program (identical on all cores)."""
    from contextlib import ExitStack

    import concourse.bass as bass
    import concourse.tile as tile
    from concourse import bacc, mybir

    f32 = mybir.dt.float32
    bf16 = mybir.dt.bfloat16
    f16 = mybir.dt.float16

    bb_count = b_per_core // bn

    nc = bacc.Bacc(
        "TRN2",
        target_bir_lowering=False,
        debug=False,
        num_devices=N_CORES,
    )

    xh = nc.dram_tensor(
        "xh", [bb_count, RH, P, J, bn], f32, kind="ExternalInput"
    ).ap()
    wt = nc.dram_tensor("wt", [K, C], f32, kind="ExternalInput").ap()
    out_t = nc.dram_tensor("out_t", [C, b_per_core], f32, kind="ExternalOutput").ap()

    wt_view = wt.rearrange("(rh p j) c -> p rh j c", rh=RH, p=P, j=J)

    scale = 1.0 / math.sqrt(K)

    WJC = 16  # j-extent of one w chunk tile
    n_wchunks = (RH * J) // WJC

    with tile.TileContext(nc) as tc, ExitStack() as ctx:
        wpool = ctx.enter_context(tc.tile_pool(name="w", bufs=1))
        wtmp_pool = ctx.enter_context(tc.tile_pool(name="wtmp", bufs=2))
        xrpool = ctx.enter_context(tc.tile_pool(name="xr", bufs=xr_bufs))
        xqpool = ctx.enter_context(tc.tile_pool(name="xq", bufs=1))
        psum_pool = ctx.enter_context(tc.tile_pool(name="psum", bufs=2, space="PSUM"))
        opool = ctx.enter_context(tc.tile_pool(name="o", bufs=2))

        # --- W prep, emitted lazily so the first x tiles interleave with
        #     W-chunk loads.
        w_tiles = [None] * n_wchunks

        def emit_wchunk(t):
            rh, j0 = (t * WJC) // J, (t * WJC) % J
            wtmp = wtmp_pool.tile([P, WJC, C], bf16)
            nc.gpsimd.dma_start(wtmp[:], wt_view[:, rh, j0 : j0 + WJC, :])
            wtile = wpool.tile([P, WJC, C], f16, tag=f"w{t}")
            nc.scalar.activation(
                wtile[:],
                wtmp[:],
                mybir.ActivationFunctionType.Sign,
                scale=float(2.0**64),
            )
            w_tiles[t] = wtile

        pending_evac = []

        def emit_evac():
            psum_e, bb_e = pending_evac.pop(0)
            ot = opool.tile([C, bn], f32)
            nc.scalar.activation(
                ot[:], psum_e[:, :], mybir.ActivationFunctionType.Copy, scale=scale
            )
            nc.sync.dma_start(out_t[:, bb_e * bn : (bb_e + 1) * bn], ot[:])

        # --- main loop: per (bb, rh), one contiguous casting DMA (split into
        #     4 j-range sub-DMAs for pipelining) straight into the matmul
        #     layout; no repack stage at all.
        JSUB = 4
        JQ = J // JSUB
        for bb in range(bb_count):
            psum = psum_pool.tile([C, bn], f32)
            for rh in range(RH):
                last = bb == bb_count - 1 and rh == RH - 1
                if last:
                    # the final tile loads as 4 SEPARATE j-quarter tiles, so
                    # each quarter's matmuls start as soon as ITS quarter
                    # lands -- only the last 32 pairs remain after the
                    # stream ends.
                    xqs = [
                        xqpool.tile([P, JQ, bn], f16, name=f"xq{s}", tag=f"xq{s}")
                        for s in range(JSUB)
                    ]
                    for s in range(JSUB):
                        nc.gpsimd.dma_start(
                            xqs[s][:],
                            xh[bb, rh, :, s * JQ : (s + 1) * JQ, :],
                        )
                    for j in range(J):
                        t = (rh * J + j) // WJC
                        nc.tensor.matmul(
                            psum[:, :],
                            w_tiles[t][:, j % WJC, :],
                            xqs[j // JQ][:, j % JQ, :],
                            start=False,
                            stop=(j == J - 1),
                        )
                    continue
                xr = xrpool.tile([P, J, bn], f16, name="xr", tag="xr")
                for s in range(JSUB):
                    j0 = s * J // JSUB
                    j1 = (s + 1) * J // JSUB
                    nc.gpsimd.dma_start(
                        xr[:, j0:j1, :],
                        xh[bb, rh, :, j0:j1, :],
                    )
                    sub_idx = (bb * RH + rh) * JSUB + s
                    if sub_idx < 8:
                        for t2 in range(sub_idx * 2, sub_idx * 2 + 2):
                            emit_wchunk(t2)
                for j in range(J):
                    t = (rh * J + j) // WJC
                    nc.tensor.matmul(
                        psum[:, :],
                        w_tiles[t][:, j % WJC, :],
                        xr[:, j, :],
                        start=(rh == 0 and j == 0),
                        stop=(rh == RH - 1 and j == J - 1),
                    )
            # evacuate with one-bb lag so the (in-order) ScalarE queue never
            # head-of-line-blocks behind this bb's matmuls.
            pending_evac.append((psum, bb))
            if len(pending_evac) > 1:
                emit_evac()
        while pending_evac:
            emit_evac()

    nc.compile()
    return nc


def _get_nc(b_per_core=B_PER_CORE, bn=128, xr_bufs=3):
    key = (b_per_core, bn, xr_bufs)
    if key not in _NC_CACHE:
        _NC_CACHE[key] = _build_nc(*key)
    return _NC_CACHE[key]


def kernel(x, W, **run_kwargs):
    from concourse import bass_utils

    x = np.asarray(x, dtype=np.float32)
    W = np.asarray(W, dtype=np.float32)
    wt = np.ascontiguousarray(W.T)  # [K, C], pure layout change

    # pure layout permutation: xh[c][bb, rh, p, j, b] = x[c*512+bb*128+b,
    # rh*(P*J) + p*J + j] -- the exact SBUF tile order, so device loads are
    # fully contiguous.
    bb_count = B_PER_CORE // 128
    x6 = x.reshape(N_CORES, bb_count, 128, RH, P, J)
    xh = np.ascontiguousarray(x6.transpose(0, 1, 3, 4, 5, 2))

    nc = _get_nc()
    in_maps = [{"xh": xh[c], "wt": wt} for c in range(N_CORES)]
    res = bass_utils.run_bass_kernel_spmd(
        nc, in_maps, core_ids=list(range(N_CORES)), **run_kwargs
    )
    out = np.concatenate([r["out_t"].T for r in res.results], axis=0)
    if run_kwargs:
        return out, res
    return out
